# revision 8
# baseline (speedup 1.0000x reference)
"""Diagonally-masked multi-head self-attention on 8 Trainium2 NeuronCores.

Problem (full shapes): x [2,2048,512], wq/wk/wv [512,512], wo [512,512],
H=8 heads, Dh=64.  out = softmax(mask_diag(q k^T / 8)) v @ wo.

Sharding: core c handles batch b = c//4 and head pair g = c%4
(heads 2g, 2g+1).  Each core computes its two heads' attention and a
partial output  y_c = sum_h (O_h / d_h) @ wo[h rows]  for its batch;
the host sums the 4 partials per batch (row-sharded wo all-reduce done
at gather time).

Per-core kernel dataflow (all matmuls bf16 with fp32 PSUM accumulation):
  xt  = x[b].T                          (uploaded pre-transposed, bf16)
  QK_h = [wq_h/8 | wk_h].T @ xt         -> [128, L]  (rows 0:64 Q^T, 64:128 K^T)
  V    = xt.T @ [wv_h0|wv_h1]           -> per key tile [128, 130] with a
                                           ones column appended per head
  S^T  = K Q^T (per 128-key tile)       -> PSUM, exp on ACT -> P^T (bf16)
  diag: P^T diagonal block zeroed via (1-I) mask multiply
  O'^T = V'^T P^T accumulated over key tiles -> [65, L] PSUM
         (row 64 = softmax denominator d, since V' col 64/129 is ones)
  y_h  = (O_h @ wo_h) * (1/d)  summed over the 2 heads on DVE.

The softmax is computed without max-subtraction: scores are ~N(0, 0.04)
(|s| < ~1.3), so exp never overflows; the diagonal -inf mask becomes a
multiply-by-zero after exp.
"""

import sys

if "/opt/trn_rl_repo" not in sys.path:
    sys.path.insert(0, "/opt/trn_rl_repo")

import numpy as np
import ml_dtypes

import concourse.bacc as bacc
import concourse.tile as tile
from concourse import mybir
from concourse.bass_utils import run_bass_kernel_spmd

N_CORES = 8
B, L, D = 2, 2048, 512
H, DH = 8, 64
HEADS_PER_CORE = 2
NKT = L // 128  # 16 key/query tiles
BF16 = mybir.dt.bfloat16
F32 = mybir.dt.float32

# test.py can flip these before calling kernel()
TRACE = False
_LAST_RESULTS = {}

_NC_CACHE = {}


def _build_nc():
    nc = bacc.Bacc(
        "TRN2",
        target_bir_lowering=False,
        debug=False,
        enable_asserts=False,
        num_devices=N_CORES,
    )
    xt = nc.dram_tensor("xt", [D, L], BF16, kind="ExternalInput").ap()
    wqk = nc.dram_tensor("wqk", [D, 256], BF16, kind="ExternalInput").ap()
    wv = nc.dram_tensor("wv", [D, 128], BF16, kind="ExternalInput").ap()
    wo = nc.dram_tensor("wo", [128, D], BF16, kind="ExternalInput").ap()
    msk = nc.dram_tensor("msk", [128, 128], BF16, kind="ExternalInput").ap()
    y = nc.dram_tensor("y", [L, D], F32, kind="ExternalOutput").ap()

    with tile.TileContext(nc) as tc:
        _emit(nc, tc, xt, wqk, wv, wo, msk, y)
    nc.compile()
    return nc


def _emit(nc, tc, xt, wqk, wv, wo, msk, y):
    import contextlib

    ctx = contextlib.ExitStack()
    with ctx:
        singles = ctx.enter_context(tc.tile_pool(name="singles", bufs=1))
        ptp = ctx.enter_context(tc.tile_pool(name="pt", bufs=3))
        ysb = ctx.enter_context(tc.tile_pool(name="ysb", bufs=4))
        psmm = ctx.enter_context(tc.tile_pool(name="psmm", bufs=2, space="PSUM"))
        psacc = ctx.enter_context(tc.tile_pool(name="psacc", bufs=1, space="PSUM"))

        # ---- loads ----
        xt_sb = []
        for c in range(4):
            t = singles.tile([128, L], BF16, tag=f"xt{c}", name=f"xt{c}")
            nc.sync.dma_start(out=t, in_=xt[c * 128 : (c + 1) * 128, :])
            xt_sb.append(t)
        wqk_sb = []
        for c in range(4):
            t = singles.tile([128, 256], BF16, tag=f"wqk{c}", name=f"wqk{c}")
            nc.sync.dma_start(out=t, in_=wqk[c * 128 : (c + 1) * 128, :])
            wqk_sb.append(t)
        wv_sb = []
        for c in range(4):
            t = singles.tile([128, 128], BF16, tag=f"wv{c}", name=f"wv{c}")
            nc.sync.dma_start(out=t, in_=wv[c * 128 : (c + 1) * 128, :])
            wv_sb.append(t)
        # wo rows per head as separate tensors (matmul operands must share
        # the same base partition, so everything lives at partition 0)
        wo_sb = []
        for h in range(2):
            t = singles.tile([64, D], BF16, tag=f"wo{h}", name=f"wo{h}")
            nc.sync.dma_start(out=t, in_=wo[h * 64 : (h + 1) * 64, :])
            wo_sb.append(t)
        msk_sb = singles.tile([128, 128], BF16, tag="msk", name="msk_sb")
        nc.sync.dma_start(out=msk_sb, in_=msk)

        # ---- QK projections: [Q^T ; K^T] = [wq_h/8 | wk_h].T @ xt ----
        # PSUM rows 0:64 are Q^T, 64:128 are K^T; both copied to
        # partition-0-based tensors (partition-shifted copy for K^T).
        q_sb = [singles.tile([64, L], BF16, tag=f"q{h}", name=f"q{h}") for h in range(2)]
        k_sb = [singles.tile([64, L], BF16, tag=f"k{h}", name=f"k{h}") for h in range(2)]
        for h in range(2):
            for nt in range(4):
                ps = psmm.tile([128, 512], F32, tag="mm", name="mm")
                for kc in range(4):
                    nc.tensor.matmul(
                        ps,
                        lhsT=wqk_sb[kc][:, h * 128 : (h + 1) * 128],
                        rhs=xt_sb[kc][:, nt * 512 : (nt + 1) * 512],
                        start=(kc == 0),
                        stop=(kc == 3),
                    )
                nc.scalar.copy(q_sb[h][:, nt * 512 : (nt + 1) * 512], ps[0:64, :])
                nc.scalar.copy(k_sb[h][:, nt * 512 : (nt + 1) * 512], ps[64:128, :])

        # ---- V projection, with ones columns at 64 and 129 ----
        v_sb = [singles.tile([128, 130], BF16, tag=f"v{lt}", name=f"v{lt}") for lt in range(NKT)]
        for lt in range(NKT):
            ps = psmm.tile([128, 128], F32, tag="mm", name="mm")
            for kc in range(4):
                nc.tensor.matmul(
                    ps,
                    lhsT=xt_sb[kc][:, lt * 128 : (lt + 1) * 128],
                    rhs=wv_sb[kc],
                    start=(kc == 0),
                    stop=(kc == 3),
                )
            nc.scalar.copy(v_sb[lt][:, 0:64], ps[:, 0:64])
            nc.scalar.copy(v_sb[lt][:, 65:129], ps[:, 64:128])
            nc.vector.memset(v_sb[lt][:, 64:65], 1.0)
            nc.vector.memset(v_sb[lt][:, 129:130], 1.0)

        # ---- attention per head ----
        ot_sb = [singles.tile([64, L], BF16, tag=f"ot{h}", name=f"ot{h}") for h in range(2)]
        drow_sb = [singles.tile([1, L], F32, tag=f"dr{h}", name=f"dr{h}") for h in range(2)]
        dt_sb = [singles.tile([128, NKT], F32, tag=f"dt{h}", name=f"dt{h}") for h in range(2)]
        rinv_sb = [singles.tile([128, NKT], F32, tag=f"ri{h}", name=f"ri{h}") for h in range(2)]
        for h in range(2):
            po = psacc.tile([65, L], F32, tag="acc", name="acc")
            for kt in range(NKT):
                pt = ptp.tile([128, L], BF16, tag="pt", name="pt")
                for j in range(2):
                    ps = psmm.tile([128, 1024], F32, tag="mm", name="mm")
                    for nt in range(2):
                        nc.tensor.matmul(
                            ps[:, nt * 512 : (nt + 1) * 512],
                            lhsT=k_sb[h][:, kt * 128 : (kt + 1) * 128],
                            rhs=q_sb[h][
                                :, j * 1024 + nt * 512 : j * 1024 + (nt + 1) * 512
                            ],
                            start=True,
                            stop=True,
                        )
                    nc.scalar.activation(
                        pt[:, j * 1024 : (j + 1) * 1024],
                        ps,
                        mybir.ActivationFunctionType.Exp,
                    )
                # zero the diagonal block (key tile kt vs query tile kt)
                nc.vector.tensor_mul(
                    pt[:, kt * 128 : (kt + 1) * 128],
                    pt[:, kt * 128 : (kt + 1) * 128],
                    msk_sb,
                )
                for nt in range(4):
                    nc.tensor.matmul(
                        po[:, nt * 512 : (nt + 1) * 512],
                        lhsT=v_sb[kt][:, h * 65 : (h + 1) * 65],
                        rhs=pt[:, nt * 512 : (nt + 1) * 512],
                        start=(kt == 0),
                        stop=(kt == NKT - 1),
                    )
            for nt in range(4):
                nc.vector.tensor_copy(
                    ot_sb[h][:, nt * 512 : (nt + 1) * 512],
                    po[0:64, nt * 512 : (nt + 1) * 512],
                )
            nc.scalar.copy(drow_sb[h], po[64:65, :])
            # respread d [1, L] -> [128, NKT] (query tile per column)
            for lt in range(NKT):
                nc.gpsimd.dma_start(
                    out=dt_sb[h][:, lt : lt + 1],
                    in_=drow_sb[h][0:1, lt * 128 : (lt + 1) * 128],
                )
            nc.vector.reciprocal(rinv_sb[h], dt_sb[h])

        # ---- output projection + per-head normalize + sum ----
        for lt in range(NKT):
            ps0 = psmm.tile([128, 512], F32, tag="mm", name="mm")
            nc.tensor.matmul(
                ps0,
                lhsT=ot_sb[0][:, lt * 128 : (lt + 1) * 128],
                rhs=wo_sb[0],
                start=True,
                stop=True,
            )
            ps1 = psmm.tile([128, 512], F32, tag="mm", name="mm")
            nc.tensor.matmul(
                ps1,
                lhsT=ot_sb[1][:, lt * 128 : (lt + 1) * 128],
                rhs=wo_sb[1],
                start=True,
                stop=True,
            )
            y0 = ysb.tile([128, 512], F32, tag="y0", name="y0")
            nc.vector.tensor_scalar_mul(y0, ps0, rinv_sb[0][:, lt : lt + 1])
            y1 = ysb.tile([128, 512], F32, tag="y1", name="y1")
            nc.vector.scalar_tensor_tensor(
                y1,
                ps1,
                rinv_sb[1][:, lt : lt + 1],
                y0,
                op0=mybir.AluOpType.mult,
                op1=mybir.AluOpType.add,
            )
            nc.sync.dma_start(out=y[lt * 128 : (lt + 1) * 128, :], in_=y1)


def _get_nc():
    if "nc" not in _NC_CACHE:
        _NC_CACHE["nc"] = _build_nc()
    return _NC_CACHE["nc"]


def kernel(x, wq, wk, wv, wo):
    x = np.asarray(x, dtype=np.float32)
    wq = np.asarray(wq, dtype=np.float32)
    wk = np.asarray(wk, dtype=np.float32)
    wv = np.asarray(wv, dtype=np.float32)
    wo = np.asarray(wo, dtype=np.float32)

    scale = 1.0 / (DH**0.5)
    bf = ml_dtypes.bfloat16
    msk = (1.0 - np.eye(128, dtype=np.float32)).astype(bf)

    in_maps = []
    for c in range(N_CORES):
        b, g = divmod(c, 4)
        h0, h1 = 2 * g, 2 * g + 1
        wqk_c = np.concatenate(
            [
                wq[:, h0 * DH : (h0 + 1) * DH] * scale,
                wk[:, h0 * DH : (h0 + 1) * DH],
                wq[:, h1 * DH : (h1 + 1) * DH] * scale,
                wk[:, h1 * DH : (h1 + 1) * DH],
            ],
            axis=1,
        )
        wv_c = wv[:, h0 * DH : (h1 + 1) * DH]
        wo_c = wo[h0 * DH : (h1 + 1) * DH, :]
        in_maps.append(
            {
                "xt": np.ascontiguousarray(x[b].T).astype(bf),
                "wqk": wqk_c.astype(bf),
                "wv": np.ascontiguousarray(wv_c).astype(bf),
                "wo": np.ascontiguousarray(wo_c).astype(bf),
                "msk": msk,
            }
        )

    nc = _get_nc()
    res = run_bass_kernel_spmd(
        nc, in_maps, core_ids=list(range(N_CORES)), trace=TRACE
    )
    _LAST_RESULTS["res"] = res

    out = np.empty((B, L, D), dtype=np.float32)
    for b in range(B):
        acc = res.results[4 * b]["y"].astype(np.float32).copy()
        for g in range(1, 4):
            acc += res.results[4 * b + g]["y"]
        out[b] = acc
    return out


# revision 10
# speedup vs baseline: 1.0306x; 1.0306x over previous
"""Diagonally-masked multi-head self-attention on 8 Trainium2 NeuronCores.

Problem (full shapes): x [2,2048,512], wq/wk/wv [512,512], wo [512,512],
H=8 heads, Dh=64.  out = softmax(mask_diag(q k^T / 8)) v @ wo.

Sharding: core c handles batch b = c//4 and head pair g = c%4
(heads 2g, 2g+1).  Each core computes its two heads' attention and a
partial output  y_c = sum_h (O_h / d_h) @ wo[h rows]  for its batch;
the host sums the 4 partials per batch (row-sharded wo all-reduce done
at gather time).

Per-core kernel dataflow (all matmuls bf16 with fp32 PSUM accumulation):
  xt  = x[b].T                          (uploaded pre-transposed, bf16)
  QK_h = [wq_h/8 | wk_h].T @ xt         -> [128, L]  (rows 0:64 Q^T, 64:128 K^T)
  V    = xt.T @ [wv_h0|wv_h1]           -> per key tile [128, 130] with a
                                           ones column appended per head
  S^T  = K Q^T (per 128-key tile)       -> PSUM, exp on ACT -> P^T (bf16)
  diag: P^T diagonal block zeroed via (1-I) mask multiply
  O'^T = V'^T P^T accumulated over key tiles -> [65, L] PSUM
         (row 64 = softmax denominator d, since V' col 64/129 is ones)
  y_h  = (O_h @ wo_h) * (1/d)  summed over the 2 heads on DVE.

The softmax is computed without max-subtraction: scores are ~N(0, 0.04)
(|s| < ~1.3), so exp never overflows; the diagonal -inf mask becomes a
multiply-by-zero after exp.
"""

import sys

if "/opt/trn_rl_repo" not in sys.path:
    sys.path.insert(0, "/opt/trn_rl_repo")

import numpy as np
import ml_dtypes

import concourse.bacc as bacc
import concourse.tile as tile
from concourse import mybir
from concourse.bass_utils import run_bass_kernel_spmd

N_CORES = 8
B, L, D = 2, 2048, 512
H, DH = 8, 64
HEADS_PER_CORE = 2
NKT = L // 128  # 16 key/query tiles
BF16 = mybir.dt.bfloat16
F32 = mybir.dt.float32

# test.py can flip these before calling kernel()
TRACE = False
_LAST_RESULTS = {}

_NC_CACHE = {}


def _build_nc():
    nc = bacc.Bacc(
        "TRN2",
        target_bir_lowering=False,
        debug=False,
        enable_asserts=False,
        num_devices=N_CORES,
    )
    xt = nc.dram_tensor("xt", [D, L], BF16, kind="ExternalInput").ap()
    wqk = nc.dram_tensor("wqk", [D, 256], BF16, kind="ExternalInput").ap()
    wv = nc.dram_tensor("wv", [D, 128], BF16, kind="ExternalInput").ap()
    wo = nc.dram_tensor("wo", [128, D], BF16, kind="ExternalInput").ap()
    msk = nc.dram_tensor("msk", [128, 128], BF16, kind="ExternalInput").ap()
    y = nc.dram_tensor("y", [L, D], F32, kind="ExternalOutput").ap()
    dscr = nc.dram_tensor("dscr", [2, L], F32, kind="Internal").ap()

    with tile.TileContext(nc) as tc:
        _emit(nc, tc, xt, wqk, wv, wo, msk, y, dscr)
    nc.compile()
    return nc


def _emit(nc, tc, xt, wqk, wv, wo, msk, y, dscr):
    import contextlib

    ctx = contextlib.ExitStack()
    with ctx:
        singles = ctx.enter_context(tc.tile_pool(name="singles", bufs=1))
        ptp = ctx.enter_context(tc.tile_pool(name="pt", bufs=4))
        ysb = ctx.enter_context(tc.tile_pool(name="ysb", bufs=4))
        psmm = ctx.enter_context(tc.tile_pool(name="psmm", bufs=2, space="PSUM"))
        psacc = ctx.enter_context(tc.tile_pool(name="psacc", bufs=2, space="PSUM"))

        # ---- loads ----
        xt_sb = []
        for c in range(4):
            t = singles.tile([128, L], BF16, tag=f"xt{c}", name=f"xt{c}")
            nc.sync.dma_start(out=t, in_=xt[c * 128 : (c + 1) * 128, :])
            xt_sb.append(t)
        wqk_sb = []
        for c in range(4):
            t = singles.tile([128, 256], BF16, tag=f"wqk{c}", name=f"wqk{c}")
            nc.sync.dma_start(out=t, in_=wqk[c * 128 : (c + 1) * 128, :])
            wqk_sb.append(t)
        wv_sb = []
        for c in range(4):
            t = singles.tile([128, 128], BF16, tag=f"wv{c}", name=f"wv{c}")
            nc.sync.dma_start(out=t, in_=wv[c * 128 : (c + 1) * 128, :])
            wv_sb.append(t)
        # wo rows per head as separate tensors (matmul operands must share
        # the same base partition, so everything lives at partition 0)
        wo_sb = []
        for h in range(2):
            t = singles.tile([64, D], BF16, tag=f"wo{h}", name=f"wo{h}")
            nc.sync.dma_start(out=t, in_=wo[h * 64 : (h + 1) * 64, :])
            wo_sb.append(t)
        msk_sb = singles.tile([128, 128], BF16, tag="msk", name="msk_sb")
        nc.sync.dma_start(out=msk_sb, in_=msk)

        # ---- QK projections: [Q^T ; K^T] = [wq_h/8 | wk_h].T @ xt ----
        # PSUM rows 0:64 are Q^T, 64:128 are K^T; both copied to
        # partition-0-based tensors (partition-shifted copy for K^T).
        q_sb = [singles.tile([64, L], BF16, tag=f"q{h}", name=f"q{h}") for h in range(2)]
        k_sb = [singles.tile([64, L], BF16, tag=f"k{h}", name=f"k{h}") for h in range(2)]
        for h in range(2):
            for nt in range(4):
                ps = psmm.tile([128, 512], F32, tag="mm", name="mm")
                for kc in range(4):
                    nc.tensor.matmul(
                        ps,
                        lhsT=wqk_sb[kc][:, h * 128 : (h + 1) * 128],
                        rhs=xt_sb[kc][:, nt * 512 : (nt + 1) * 512],
                        start=(kc == 0),
                        stop=(kc == 3),
                    )
                nc.vector.tensor_copy(q_sb[h][:, nt * 512 : (nt + 1) * 512], ps[0:64, :])
                nc.vector.tensor_copy(k_sb[h][:, nt * 512 : (nt + 1) * 512], ps[64:128, :])

        # ---- V projection, with ones columns at 64 and 129 ----
        v_sb = [singles.tile([128, 130], BF16, tag=f"v{lt}", name=f"v{lt}") for lt in range(NKT)]
        for lt in range(NKT):
            ps = psmm.tile([128, 128], F32, tag="mm", name="mm")
            for kc in range(4):
                nc.tensor.matmul(
                    ps,
                    lhsT=xt_sb[kc][:, lt * 128 : (lt + 1) * 128],
                    rhs=wv_sb[kc],
                    start=(kc == 0),
                    stop=(kc == 3),
                )
            nc.vector.tensor_copy(v_sb[lt][:, 0:64], ps[:, 0:64])
            nc.vector.tensor_copy(v_sb[lt][:, 65:129], ps[:, 64:128])
            nc.vector.memset(v_sb[lt][:, 64:65], 1.0)
            nc.vector.memset(v_sb[lt][:, 129:130], 1.0)

        # ---- attention, query range split in halves for PSUM headroom ----
        ot_sb = [singles.tile([64, L], BF16, tag=f"ot{h}", name=f"ot{h}") for h in range(2)]
        drow_sb = [singles.tile([1, L], F32, tag=f"dr{h}", name=f"dr{h}") for h in range(2)]
        dt_sb = [singles.tile([128, NKT], F32, tag=f"dt{h}", name=f"dt{h}") for h in range(2)]
        rinv_sb = [singles.tile([128, NKT], F32, tag=f"ri{h}", name=f"ri{h}") for h in range(2)]
        HQ = L // 2  # 1024 queries per half
        for h in range(2):
            for hf in range(2):
                po = psacc.tile([65, HQ], F32, tag="acc", name="acc")
                for kt in range(NKT):
                    pt = ptp.tile([128, HQ], BF16, tag="pt", name="pt")
                    ps = psmm.tile([128, HQ], F32, tag="mm", name="mm")
                    for nt in range(2):
                        nc.tensor.matmul(
                            ps[:, nt * 512 : (nt + 1) * 512],
                            lhsT=k_sb[h][:, kt * 128 : (kt + 1) * 128],
                            rhs=q_sb[h][
                                :, hf * HQ + nt * 512 : hf * HQ + (nt + 1) * 512
                            ],
                            start=True,
                            stop=True,
                        )
                    nc.scalar.activation(
                        pt, ps, mybir.ActivationFunctionType.Exp
                    )
                    # zero the diagonal block when it falls in this half
                    if kt // 8 == hf:
                        off = (kt % 8) * 128
                        nc.vector.tensor_mul(
                            pt[:, off : off + 128],
                            pt[:, off : off + 128],
                            msk_sb,
                        )
                    for nt in range(2):
                        nc.tensor.matmul(
                            po[:, nt * 512 : (nt + 1) * 512],
                            lhsT=v_sb[kt][:, h * 65 : (h + 1) * 65],
                            rhs=pt[:, nt * 512 : (nt + 1) * 512],
                            start=(kt == 0),
                            stop=(kt == NKT - 1),
                        )
                nc.vector.tensor_copy(ot_sb[h][:, hf * HQ : (hf + 1) * HQ], po[0:64, :])
                nc.vector.tensor_copy(
                    drow_sb[h][:, hf * HQ : (hf + 1) * HQ], po[64:65, :]
                )
            # d respread [1, L] -> [128, NKT] via a DRAM bounce (DMA cannot
            # cross partitions from a 1-partition SBUF source in one AP)
            nc.sync.dma_start(out=dscr[h : h + 1, :], in_=drow_sb[h])
            nc.sync.dma_start(
                out=dt_sb[h],
                in_=dscr[h, :].rearrange("(l p) -> p l", p=128),
            )
            nc.vector.reciprocal(rinv_sb[h], dt_sb[h])

        # ---- output projection + per-head normalize + sum ----
        for lt in range(NKT):
            ps0 = psmm.tile([128, 512], F32, tag="mm", name="mm")
            nc.tensor.matmul(
                ps0,
                lhsT=ot_sb[0][:, lt * 128 : (lt + 1) * 128],
                rhs=wo_sb[0],
                start=True,
                stop=True,
            )
            ps1 = psmm.tile([128, 512], F32, tag="mm", name="mm")
            nc.tensor.matmul(
                ps1,
                lhsT=ot_sb[1][:, lt * 128 : (lt + 1) * 128],
                rhs=wo_sb[1],
                start=True,
                stop=True,
            )
            y0 = ysb.tile([128, 512], F32, tag="y0", name="y0")
            nc.vector.tensor_scalar_mul(y0, ps0, rinv_sb[0][:, lt : lt + 1])
            y1 = ysb.tile([128, 512], F32, tag="y1", name="y1")
            nc.vector.scalar_tensor_tensor(
                y1,
                ps1,
                rinv_sb[1][:, lt : lt + 1],
                y0,
                op0=mybir.AluOpType.mult,
                op1=mybir.AluOpType.add,
            )
            nc.sync.dma_start(out=y[lt * 128 : (lt + 1) * 128, :], in_=y1)


def _get_nc():
    if "nc" not in _NC_CACHE:
        _NC_CACHE["nc"] = _build_nc()
    return _NC_CACHE["nc"]


def kernel(x, wq, wk, wv, wo):
    x = np.asarray(x, dtype=np.float32)
    wq = np.asarray(wq, dtype=np.float32)
    wk = np.asarray(wk, dtype=np.float32)
    wv = np.asarray(wv, dtype=np.float32)
    wo = np.asarray(wo, dtype=np.float32)

    scale = 1.0 / (DH**0.5)
    bf = ml_dtypes.bfloat16
    msk = (1.0 - np.eye(128, dtype=np.float32)).astype(bf)

    in_maps = []
    for c in range(N_CORES):
        b, g = divmod(c, 4)
        h0, h1 = 2 * g, 2 * g + 1
        wqk_c = np.concatenate(
            [
                wq[:, h0 * DH : (h0 + 1) * DH] * scale,
                wk[:, h0 * DH : (h0 + 1) * DH],
                wq[:, h1 * DH : (h1 + 1) * DH] * scale,
                wk[:, h1 * DH : (h1 + 1) * DH],
            ],
            axis=1,
        )
        wv_c = wv[:, h0 * DH : (h1 + 1) * DH]
        wo_c = wo[h0 * DH : (h1 + 1) * DH, :]
        in_maps.append(
            {
                "xt": np.ascontiguousarray(x[b].T).astype(bf),
                "wqk": wqk_c.astype(bf),
                "wv": np.ascontiguousarray(wv_c).astype(bf),
                "wo": np.ascontiguousarray(wo_c).astype(bf),
                "msk": msk,
            }
        )

    nc = _get_nc()
    res = run_bass_kernel_spmd(
        nc, in_maps, core_ids=list(range(N_CORES)), trace=TRACE
    )
    _LAST_RESULTS["res"] = res

    out = np.empty((B, L, D), dtype=np.float32)
    for b in range(B):
        acc = res.results[4 * b]["y"].astype(np.float32).copy()
        for g in range(1, 4):
            acc += res.results[4 * b + g]["y"]
        out[b] = acc
    return out
